# revision 1
# baseline (speedup 1.0000x reference)
"""Trainium2 Bass kernel for nn_Linear_6373731467798 (binarized dense layer).

Math (forward values only):
    act   = sign(x + bias)                      # +-1 (0 on exact zero)
    scale = mean(|weight|)
    w_eff = scale * sign(weight)
    out   = act @ w_eff.T = scale * (sign(x+bias) @ sign(weight).T)

Both matmul operands are in {-1, 0, +1}, exactly representable in fp8, and
PSUM accumulates in fp32, so the low-precision matmul is numerically exact;
the single fp32 `scale` multiply is applied on the way out of PSUM. The
output is stored as fp16: out = scale * k with k an integer |k| <= 1024, so
fp16 round-off is <= 2^-11 relative — far inside the accuracy budget — and
it halves the output HBM traffic.

Sharding: data-parallel over 8 NeuronCores along the N=32768 batch dim
(4096 rows/core); bias and the binarized weight are replicated. No
collectives needed (forward only).

Per-core pipeline. The kernel is limited by the PE engine (fp8 DoubleRow
matmuls + the 128x128 fp32 x-transposes) and by DMA (29.4 MB at ~360 GB/s),
which are close to balanced. Two scheduling points matter:
  - transposes run in PE transpose-mode, which does not count as "busy" for
    the PE's HAM clock gate: long transpose-only bursts let the clock drop
    to 1.2 GHz and each following matmul burst restarts cold. The main loop
    therefore software-pipelines groups: group g's transposes+signs are
    interleaved 1:1 with group g-1's matmul packs so the PE alternates at a
    fine grain and the clock stays warm.
  - DMA issue is spread over the SP (x loads), Pool (weight load, out
    stores) queues so issue overhead and transfers overlap.
  main loop (4 groups of 8 row-tiles = 1024 rows):
    stage(g):   per k-block b: 8 PE fp32-transposes into a 2-bank PSUM
                tile, then 2 ACT ops: sign(x_T + bias_b) -> fp8 act_T
    compute(g): per row-tile j: 8 fp8 DoubleRow matmuls -> PSUM;
                DVE: out = psum * scale -> fp16 SBUF; one out DMA per group
"""

import sys

for _p in ("/opt/trn_rl_repo",):
    if _p not in sys.path:
        sys.path.insert(0, _p)

import numpy as np

import concourse.bass as bass
import concourse.tile as tile
from concourse import bacc, mybir

N = 32768
D = 1024
NCORES = 8
NSHARD = N // NCORES  # 4096
P = 128
NB = D // P  # 8 contraction blocks
GN = 8  # row-tiles per DMA group (1024 rows / 4 MB per x DMA)
NGROUP = NSHARD // (GN * P)  # 4 groups
F32 = mybir.dt.float32
F16 = mybir.dt.float16
BF16 = mybir.dt.bfloat16
FP8 = mybir.dt.float8e4

MM_DT = FP8
OUT_DT = F16


def build_program(num_cores: int = NCORES, reps: int = 1, full: bool = False) -> bass.Bass:
    from contextlib import ExitStack

    from concourse.masks import make_identity

    nc = bacc.Bacc(
        "TRN2",
        target_bir_lowering=False,
        debug=False,
        enable_asserts=True,
        num_devices=num_cores,
    )

    x_ap = nc.dram_tensor("x", [NSHARD, D], F32, kind="ExternalInput").ap()
    b_ap = nc.dram_tensor("bias", [D], F32, kind="ExternalInput").ap()
    w_ap = nc.dram_tensor("weight", [D, D], F32, kind="ExternalInput").ap()
    o_ap = nc.dram_tensor("out", [NSHARD, D], OUT_DT, kind="ExternalOutput").ap()

    with tile.TileContext(nc) as tc, ExitStack() as ctx:
        const = ctx.enter_context(tc.tile_pool(name="const", bufs=1))
        wt_pool = ctx.enter_context(tc.tile_pool(name="wT", bufs=1))

        ident_f = const.tile([P, P], F32, tag="ident_f")
        make_identity(nc, ident_f[:])
        ident_b = const.tile([P, P], BF16, tag="ident_b")
        make_identity(nc, ident_b[:])

        # bias_sb[p, b] = bias[b*128 + p]  (per-partition bias per i-block)
        bias_sb = const.tile([P, NB], F32, tag="bias")
        nc.sync.dma_start(out=bias_sb[:], in_=b_ap.rearrange("(b p) -> p b", p=P))

        ones_col = const.tile([P, 1], F32, tag="ones_col")
        nc.vector.memset(ones_col[:], 1.0)
        ones_row = const.tile([1, P], F32, tag="ones_row")
        nc.vector.memset(ones_row[:], 1.0)

        # xpool holds 4-MB tiles; the weight stage borrows a slot in the
        # prologue (same shape), so SBUF stays within budget at bufs=3.
        xpool = ctx.enter_context(tc.tile_pool(name="x", bufs=3))
        apool = ctx.enter_context(tc.tile_pool(name="actT", bufs=3))
        opool = ctx.enter_context(tc.tile_pool(name="o", bufs=3))
        wstage = ctx.enter_context(tc.tile_pool(name="wstage", bufs=1))
        psum_x = ctx.enter_context(tc.tile_pool(name="psum_x", bufs=2, space="PSUM"))
        psum_mm = ctx.enter_context(tc.tile_pool(name="psum_mm", bufs=2, space="PSUM"))

        for _rep in range(reps if full else 1):
            # ---- weight DMA first: its dependent chain (sign -> transpose
            # -> first matmuls of this rep) is longer than the x chain ----
            # wfull[p, t, :] = weight[t*128 + p, :]
            wfull = xpool.tile([P, NB, D], F32, tag="x")
            nc.gpsimd.dma_start(
                out=wfull[:], in_=w_ap.rearrange("(t p) i -> p t i", p=P)
            )

            x_first = xpool.tile([P, GN, D], F32, tag="x")
            rows0 = slice(0, GN * P)
            nc.sync.dma_start(
                out=x_first[:], in_=x_ap[rows0, :].rearrange("(a p) i -> p a i", p=P)
            )

            # all signs first: the PE's W-transposes gate on them, while the
            # |w| sums (below) are needed much later, by the scale chain
            wsg = opool.tile([P, NB, D], BF16, tag="o")
            asum = wstage.tile([P, NB], F32, tag="asum")
            for t in range(NB):
                nc.scalar.sign(wsg[:, t, :], wfull[:, t, :])
            for t in range(NB):
                wscr = wstage.tile([P, D], BF16, tag="wscr", bufs=2)
                nc.scalar.activation(
                    wscr[:],
                    wfull[:, t, :],
                    mybir.ActivationFunctionType.Abs,
                    accum_out=asum[:, t : t + 1],
                )

            # W_T[p2, b, o] = sign(weight)[o, b*128+p2]   (fp8, resident)
            w_t = wt_pool.tile([P, NB, D], MM_DT, tag="wT")
            for t in range(NB):
                pwt = psum_mm.tile([P, NB, P], BF16, tag="mm")
                for b in range(NB):
                    nc.tensor.transpose(
                        pwt[:, b, :],
                        wsg[:, t, b * P : (b + 1) * P],
                        ident_b[:],
                    )
                nc.vector.tensor_copy(w_t[:, :, t * P : (t + 1) * P], pwt[:])

            def emit_scale_chain():
                # reduce asum over its NB columns (Abs is identity on >=0)
                colsum = wstage.tile([P, 1], F32, tag="colsum")
                ascr = wstage.tile([P, NB], BF16, tag="ascr")
                nc.scalar.activation(
                    ascr[:],
                    asum[:],
                    mybir.ActivationFunctionType.Abs,
                    accum_out=colsum[:],
                )
                # partition reduce + broadcast via ones-matmuls
                tot_ps = psum_mm.tile([1, 1], F32, tag="mm")
                nc.tensor.matmul(
                    tot_ps[:], ones_col[:], colsum[:], start=True, stop=True
                )
                tot_sb = wstage.tile([1, 1], F32, tag="tot")
                nc.vector.tensor_copy(tot_sb[:], tot_ps[:])
                bcast_ps = psum_mm.tile([P, 1], F32, tag="mm")
                nc.tensor.matmul(
                    bcast_ps[:], ones_row[:], tot_sb[:], start=True, stop=True
                )
                scale_sb = wstage.tile([P, 1], F32, tag="scale")
                nc.vector.tensor_scalar_mul(scale_sb[:], bcast_ps[:], 1.0 / (D * D))
                return scale_sb

            scale_sb = None  # emitted after group 0 is staged: its two tiny
            # PE matmuls wait on the ACT |W| chain and would otherwise
            # head-of-line-block the first group's transposes in the PE queue

            # ---- main loop: software-pipelined stage/compute ----
            niter = NGROUP * (1 if full else reps)

            def stage_b(x_sb, act_t, b):
                """Transpose k-block b of all GN row-tiles + sign to fp8."""
                pt = psum_x.tile([P, GN, P], F32, tag="xtr")
                for j in range(GN):
                    nc.tensor.transpose(
                        pt[:, j, :],
                        x_sb[:, j, b * P : (b + 1) * P],
                        ident_f[:],
                    )
                for h in range(2):
                    nc.scalar.sign(
                        act_t[:, b, h * (GN // 2) : (h + 1) * (GN // 2), :],
                        pt[:, h * (GN // 2) : (h + 1) * (GN // 2), :],
                        bias=bias_sb[:, b : b + 1],
                    )

            def compute_j(g, act_t, o_sb, j):
                """Matmuls + scale for row-tile j; out DMA after the last."""
                po = psum_mm.tile([P, D], F32, tag="mm")
                for c in range(NB // 2):
                    for h2 in range(2):
                        nc.tensor.matmul(
                            po[:, h2 * 512 : (h2 + 1) * 512],
                            act_t[:, 2 * c : 2 * c + 2, j, :],
                            w_t[:, 2 * c : 2 * c + 2, h2 * 512 : (h2 + 1) * 512],
                            start=(c == 0),
                            stop=(c == NB // 2 - 1),
                            perf_mode=mybir.MatmulPerfMode.DoubleRow,
                        )
                for h2 in range(2):
                    nc.vector.tensor_scalar_mul(
                        o_sb[:, j, h2 * 512 : (h2 + 1) * 512],
                        po[:, h2 * 512 : (h2 + 1) * 512],
                        scale_sb[:],
                    )
                if j == GN - 1:
                    rows = slice(g * GN * P, (g + 1) * GN * P)
                    nc.gpsimd.dma_start(
                        out=o_ap[rows, :].rearrange("(a p) i -> p a i", p=P),
                        in_=o_sb[:],
                    )

            prev = None  # (g, act_t, o_sb) of the staged-but-not-computed group
            x_tiles = {0: x_first}
            for it in range(niter):
                # prefetch the NEXT group's x one full step ahead so this
                # step's transposes never head-of-line-block the PE queue
                if it + 1 < niter:
                    nxt = xpool.tile([P, GN, D], F32, tag="x")
                    gn = (it + 1) % NGROUP
                    nc.sync.dma_start(
                        out=nxt[:],
                        in_=x_ap[gn * GN * P : (gn + 1) * GN * P, :].rearrange(
                            "(a p) i -> p a i", p=P
                        ),
                    )
                    x_tiles[it + 1] = nxt
                g = it % NGROUP
                x_sb = x_tiles.pop(it)
                act_t = apool.tile([P, NB, GN, P], MM_DT, tag="actT")
                o_sb = opool.tile([P, GN, D], OUT_DT, tag="o")
                # interleave: stage k-block b of group `it`, then matmul pack
                # j=b of the previous group — keeps the PE warm throughout
                for b in range(NB):
                    stage_b(x_sb, act_t, b)
                    if prev is not None:
                        compute_j(prev[0], prev[1], prev[2], b)
                if scale_sb is None:
                    scale_sb = emit_scale_chain()
                prev = (g, act_t, o_sb)
            # drain: compute the last staged group
            for j in range(GN):
                compute_j(prev[0], prev[1], prev[2], j)

    nc.compile()
    return nc


_PROGRAM_CACHE: dict[int, bass.Bass] = {}


def _get_program(num_cores: int = NCORES) -> bass.Bass:
    if num_cores not in _PROGRAM_CACHE:
        _PROGRAM_CACHE[num_cores] = build_program(num_cores)
    return _PROGRAM_CACHE[num_cores]


def kernel(x: np.ndarray, bias: np.ndarray, weight: np.ndarray) -> np.ndarray:
    from concourse.bass_utils import run_bass_kernel_spmd

    x = np.ascontiguousarray(np.asarray(x, dtype=np.float32))
    bias = np.ascontiguousarray(np.asarray(bias, dtype=np.float32))
    weight = np.ascontiguousarray(np.asarray(weight, dtype=np.float32))
    assert x.shape == (N, D) and bias.shape == (D,) and weight.shape == (D, D)

    nc = _get_program(NCORES)
    in_maps = [
        {"x": x[c * NSHARD : (c + 1) * NSHARD], "bias": bias, "weight": weight}
        for c in range(NCORES)
    ]
    res = run_bass_kernel_spmd(nc, in_maps, list(range(NCORES)))
    out = np.concatenate([res.results[c]["out"] for c in range(NCORES)], axis=0)
    return out.astype(np.float32)

